# revision 2
# baseline (speedup 1.0000x reference)
"""Distributed Trainium2 kernel for AlternateWeaveGather (segment_reduce).

Reference computation:
    h = x @ W.T + b                      # [N, 512] linear
    out = segment_mean(h, batch, 256)    # [256, 512]

The linear commutes with the segment sum, so each core only segment-
reduces its row shard of x (one-hot matmuls on the TensorEngine) and the
tiny 512x512 linear runs once per owned segment block at the end:
    out[s] = (segsum_x[s] @ W.T) * inv[s] + b * (cnt[s] > 0)
with inv/cnt host-derived from batch (metadata, like the index tensors).

batch is sorted, so core j's 16384 rows span ~33 contiguous segments and
cross-core contributions exist only at the shard boundaries, confined to
each core's first/last 2048 rows (host-asserted). The kernel therefore
processes supertiles {0, 7} first into accumulator A, extracts the <=16
boundary-partial rows with a selection matmul, and launches a tiny
AllGather (16KB/core) that completes while supertiles 1..6 stream into
accumulator B. The epilogue combines A+B+gathered partials with
selection matmuls (inv-counts folded into the selection weights) - no
big collective, no indirect scatter, ~3us serial tail.

Sharding: data-parallel over rows. x/batch split along dim 0 across 8
cores; W/b replicated; host concatenates the 8x[32, 512] outputs.
"""

import numpy as np

import concourse.bacc as bacc
import concourse.bass as bass
import concourse.mybir as mybir
import concourse.tile as tile
from concourse.bass_utils import run_bass_kernel_spmd

N_CORES = 8
N_ROWS = 131072
D = 512
N_SEG = 256
SEG_PER_CORE = N_SEG // N_CORES  # 32
RPC = N_ROWS // N_CORES          # 16384 rows per core
K = 16                           # row planes per supertile
SUP = 128 * K                    # 2048 rows per supertile
N_SUP = RPC // SUP               # 8
W_WIN = 128                      # one-hot window width (span is ~33)
EXP = 8                          # exported segments per side
NEX = 2 * EXP                    # 16 exchange slots per core

F32 = mybir.dt.float32
I32 = mybir.dt.int32
BF16 = mybir.dt.bfloat16

# supertile processing order: boundary tiles first (accumulator A),
# interior tiles after (accumulator B)
A_TILES = [0, N_SUP - 1]
B_TILES = list(range(1, N_SUP - 1))


def build_nc():
    nc = bacc.Bacc("TRN2", target_bir_lowering=False, debug=False,
                   num_devices=N_CORES)
    x = nc.dram_tensor("x", [RPC, D], F32, kind="ExternalInput")
    # batchp[p, k*N_SUP + t] = batch_rel[SUP*t + K*p + k]
    batchp = nc.dram_tensor("batchp", [128, K * N_SUP], F32,
                            kind="ExternalInput")
    iota = nc.dram_tensor("iota", [128, W_WIN], BF16, kind="ExternalInput")
    selx = nc.dram_tensor("selx", [128, NEX], BF16, kind="ExternalInput")
    sell = nc.dram_tensor("sell", [128, SEG_PER_CORE], BF16,
                          kind="ExternalInput")
    selg = nc.dram_tensor("selg", [128, SEG_PER_CORE], BF16,
                          kind="ExternalInput")
    wt = nc.dram_tensor("wt", [D, D], BF16, kind="ExternalInput")
    bindb = nc.dram_tensor("bindb", [SEG_PER_CORE, D], F32,
                           kind="ExternalInput")
    out = nc.dram_tensor("out", [SEG_PER_CORE, D], F32, kind="ExternalOutput")

    warm_z = nc.inline_tensor(np.zeros((128, 2), dtype=np.float32),
                              name="warm_z")

    # [N_SUP, 128, K, 512]; per (t, p) the (K, 512) block is 32KB contiguous
    x_r = x.ap().rearrange("(t p k) d -> t p k d", p=128, k=K)
    RG = [list(range(N_CORES))]

    with tile.TileContext(nc) as tc:
        with tc.tile_pool(name="const", bufs=1) as const, \
             tc.tile_pool(name="dram", bufs=1, space="DRAM") as dram:
            iota_sb = const.tile([128, W_WIN], BF16, name="iota_sb")
            batch_sb = const.tile([128, K * N_SUP], F32, name="batch_sb")
            selx_sb = const.tile([128, NEX], BF16, name="selx_sb")
            sell_sb = const.tile([128, SEG_PER_CORE], BF16, name="sell_sb")
            selg_sb = const.tile([128, SEG_PER_CORE], BF16, name="selg_sb")
            wt_sb = const.tile([128, 4 * D], BF16, name="wt_sb")
            bind_sb = const.tile([SEG_PER_CORE, D], F32, name="bind_sb")
            a_sb = const.tile([128, D], BF16, name="a_sb")
            b_sb = const.tile([128, D], BF16, name="b_sb")
            ex_sb = const.tile([NEX, D], BF16, name="ex_sb")
            g_sb = const.tile([128, D], BF16, name="g_sb")
            lhsT = const.tile([128, 4 * SEG_PER_CORE], BF16, name="lhsT")
            res = const.tile([SEG_PER_CORE, D], F32, name="res")

            ag_in = dram.tile([NEX, D], BF16, name="ag_in")
            ag_out = dram.tile([N_CORES * NEX, D], BF16, name="ag_out",
                               addr_space="Shared")

            # fire-and-forget tiny AllReduce: warms the ncfw collective
            # path (cold doorbell->poll costs ~12us) while the main loop
            # streams; nothing depends on its output
            warm_in = dram.tile([128, 2], F32, name="warm_in")
            warm_out = dram.tile([128, 2], F32, name="warm_out",
                                 addr_space="Shared")
            warm_dma = nc.gpsimd.dma_start(out=warm_in[:, :],
                                           in_=warm_z[:, :])
            warm_cc = nc.gpsimd.collective_compute(
                "AllReduce", mybir.AluOpType.add, replica_groups=RG,
                ins=[warm_in.opt()], outs=[warm_out.opt()])
            bass._add_dep_helper(warm_cc.ins, warm_dma.ins, False,
                                 "warm AR right after its input")

            # small inputs via SWDGE; keeps the HWDGE x queues clean
            nc.gpsimd.dma_start(out=iota_sb[:, :], in_=iota[:, :])
            nc.gpsimd.dma_start(out=batch_sb[:, :], in_=batchp[:, :])
            nc.gpsimd.dma_start(out=selx_sb[:, :], in_=selx[:, :])
            nc.gpsimd.dma_start(out=sell_sb[:, :], in_=sell[:, :])
            nc.gpsimd.dma_start(out=selg_sb[:, :], in_=selg[:, :])
            for i in range(4):
                nc.gpsimd.dma_start(out=wt_sb[:, i * D:(i + 1) * D],
                                    in_=wt[i * 128:(i + 1) * 128, :])
            nc.gpsimd.dma_start(out=bind_sb[:, :], in_=bindb[:, :])

            with tc.tile_pool(name="xin", bufs=4) as xp, \
                 tc.tile_pool(name="ohp", bufs=12) as ohp, \
                 tc.tile_pool(name="pacc", bufs=1, space="PSUM") as pacc, \
                 tc.tile_pool(name="pepi", bufs=1, space="PSUM") as pepi:
                psA = pacc.tile([128, D], F32, name="psA")
                psB = pacc.tile([128, D], F32, name="psB")
                psE = pepi.tile([NEX, D], F32, name="psE")
                pt = [pepi.tile([128, SEG_PER_CORE], F32, name=f"pt{c}")
                      for c in range(4)]
                po = pepi.tile([SEG_PER_CORE, D], F32, name="po")

                def stream_tile(t, ps, start, stop, split4=False):
                    xt = xp.tile([128, K, D], F32, name="xt")
                    if split4:
                        # plane-chunked: fine drain granularity at the end
                        for c in range(4):
                            q = nc.sync if c % 2 == 0 else nc.scalar
                            q.dma_start(out=xt[:, 4 * c:4 * c + 4, :],
                                        in_=x_r[t][:, 4 * c:4 * c + 4, :])
                    else:
                        # partition-split: 32KB contiguous per-partition
                        # descriptors, each queue feeds its own SDMA set
                        nc.sync.dma_start(out=xt[0:64, :, :],
                                          in_=x_r[t][0:64, :, :])
                        nc.scalar.dma_start(out=xt[64:128, :, :],
                                            in_=x_r[t][64:128, :, :])
                    # little-endian f32: the high halfword of each element
                    # IS its truncated bf16 value -> free bf16 operand
                    xt_bf = xt[:, :, :].bitcast(BF16)
                    for k in range(K):
                        oh = ohp.tile([128, W_WIN], BF16, name="oh")
                        nc.vector.tensor_scalar(
                            oh[:, :], iota_sb[:, :],
                            batch_sb[:, k * N_SUP + t:k * N_SUP + t + 1],
                            None, mybir.AluOpType.is_equal)
                        nc.tensor.matmul(ps[:, :], oh[:, :],
                                         xt_bf[:, k, 1::2],
                                         start=(start and k == 0),
                                         stop=(stop and k == K - 1),
                                         skip_group_check=True)

                # phase A: boundary supertiles -> psA
                for i, t in enumerate(A_TILES):
                    stream_tile(t, psA, start=(i == 0),
                                stop=(i == len(A_TILES) - 1))

                # boundary exchange: extract export rows from A, AllGather
                nc.vector.tensor_copy(a_sb[:, :], psA[:, :])
                nc.tensor.matmul(psE[:, :], selx_sb[:, :], a_sb[:, :],
                                 start=True, stop=True,
                                 skip_group_check=True)
                nc.vector.tensor_copy(ex_sb[:, :], psE[:, :])
                nc.gpsimd.dma_start(out=ag_in[:, :], in_=ex_sb[:, :])
                nc.gpsimd.collective_compute(
                    "AllGather", mybir.AluOpType.bypass, replica_groups=RG,
                    ins=[ag_in.opt()], outs=[ag_out.opt()])
                nc.gpsimd.dma_start(out=g_sb[:, :], in_=ag_out[:, :])

                # hoist the A-part of the owned-segment selection
                for c in range(4):
                    nc.tensor.matmul(pt[c][:, :],
                                     a_sb[:, c * 128:(c + 1) * 128],
                                     sell_sb[:, :], start=True, stop=False,
                                     skip_group_check=True)

                # phase B: interior supertiles -> psB
                for i, t in enumerate(B_TILES):
                    last = i == len(B_TILES) - 1
                    stream_tile(t, psB, start=(i == 0), stop=last,
                                split4=last)
                    if t == B_TILES[-2]:
                        # gathered-part selection (g_sb ready mid-stream)
                        for c in range(4):
                            nc.tensor.matmul(
                                pt[c][:, :], g_sb[:, c * 128:(c + 1) * 128],
                                selg_sb[:, :], start=False, stop=False,
                                skip_group_check=True)

                # tail: B-part selection, transpose-combine, tiny linear
                nc.vector.tensor_copy(b_sb[:, :], psB[:, :])
                for c in range(4):
                    nc.tensor.matmul(pt[c][:, :],
                                     b_sb[:, c * 128:(c + 1) * 128],
                                     sell_sb[:, :], start=False, stop=True,
                                     skip_group_check=True)
                for c in range(4):
                    nc.vector.tensor_copy(
                        lhsT[:, c * SEG_PER_CORE:(c + 1) * SEG_PER_CORE],
                        pt[c][:, :])
                for c in range(4):
                    nc.tensor.matmul(
                        po[:, :],
                        lhsT[:, c * SEG_PER_CORE:(c + 1) * SEG_PER_CORE],
                        wt_sb[:, c * D:(c + 1) * D],
                        start=(c == 0), stop=(c == 3),
                        skip_group_check=True)
                # res = seg_mean @ Wt + b*(cnt>0)  (inv folded into sel)
                nc.vector.tensor_tensor(res[:, :], po[:, :], bind_sb[:, :],
                                        mybir.AluOpType.add)
                nc.sync.dma_start(out=out[:, :], in_=res[:, :])
    nc.compile()
    return nc


def make_in_maps(x, W, b, batch):
    x = np.asarray(x, dtype=np.float32)
    W = np.asarray(W, dtype=np.float32)
    b = np.asarray(b, dtype=np.float32)
    batch = np.asarray(batch).astype(np.int64)
    npbf = mybir.dt.np(BF16)
    wt = np.ascontiguousarray(W.T).astype(npbf)
    iota = np.tile(np.arange(W_WIN, dtype=np.float32), (128, 1)).astype(npbf)
    counts = np.bincount(batch, minlength=N_SEG).astype(np.float64)
    inv = (1.0 / np.maximum(counts, 1.0)).astype(np.float32)
    ind = (counts > 0).astype(np.float32)

    def slot_seg(j, s):
        # exchange slot s of core j: 8 segs below / above its owned block
        if s < EXP:
            return SEG_PER_CORE * j - EXP + s
        return SEG_PER_CORE * (j + 1) + (s - EXP)

    in_maps = []
    for j in range(N_CORES):
        bs = batch[j * RPC:(j + 1) * RPC]
        base, last = int(bs[0]), int(bs[-1])
        lo, hi = SEG_PER_CORE * j, SEG_PER_CORE * (j + 1)
        rel = (bs - base).astype(np.float32)
        assert rel.max() < W_WIN, f"core {j}: span {int(rel.max()) + 1}"
        assert base >= lo - EXP and last < hi + EXP, \
            f"core {j}: boundary outside +-{EXP} margin ({base}, {last})"
        # foreign-segment rows must sit in the first/last supertile
        assert np.searchsorted(bs, lo) <= SUP, f"core {j}: low margin"
        assert RPC - np.searchsorted(bs, hi) <= SUP, f"core {j}: high margin"

        planes = rel.reshape(N_SUP, 128, K)
        bp = np.empty((128, K * N_SUP), np.float32)
        for k in range(K):
            bp[:, k * N_SUP:(k + 1) * N_SUP] = planes[:, :, k].T

        selx = np.zeros((128, NEX), np.float32)
        for s in range(NEX):
            seg = slot_seg(j, s)
            if 0 <= seg < N_SEG and 0 <= seg - base < W_WIN:
                selx[seg - base, s] = 1.0
        sell = np.zeros((128, SEG_PER_CORE), np.float32)
        for u in range(SEG_PER_CORE):
            p = lo + u - base
            if 0 <= p < W_WIN:
                sell[p, u] = inv[lo + u]
        selg = np.zeros((N_CORES * NEX, SEG_PER_CORE), np.float32)
        for r in range(N_CORES):
            if r == j:
                continue
            for s in range(NEX):
                seg = slot_seg(r, s)
                if lo <= seg < hi:
                    selg[r * NEX + s, seg - lo] = inv[seg]
        bindb = b.reshape(1, D) * ind[lo:hi, None]

        in_maps.append({
            "x": np.ascontiguousarray(x[j * RPC:(j + 1) * RPC]),
            "batchp": np.ascontiguousarray(bp),
            "iota": iota,
            "selx": np.ascontiguousarray(selx.astype(npbf)),
            "sell": np.ascontiguousarray(sell.astype(npbf)),
            "selg": np.ascontiguousarray(selg.astype(npbf)),
            "wt": wt,
            "bindb": np.ascontiguousarray(bindb.astype(np.float32)),
        })
    return in_maps


_NC_CACHE = {}


def kernel(x, W, b, batch, num_segments, trace=False):
    assert int(num_segments) == N_SEG
    if "nc" not in _NC_CACHE:
        _NC_CACHE["nc"] = build_nc()
    nc = _NC_CACHE["nc"]
    in_maps = make_in_maps(x, W, b, batch)
    res = run_bass_kernel_spmd(nc, in_maps, core_ids=list(range(N_CORES)),
                               trace=trace)
    full = np.concatenate([res.results[j]["out"] for j in range(N_CORES)],
                          axis=0)
    if trace:
        return full, res
    return full


# revision 6
# speedup vs baseline: 1.4620x; 1.4620x over previous
"""Distributed Trainium2 kernel for AlternateWeaveGather (segment_reduce).

Reference computation:
    h = x @ W.T + b                      # [N, 512] linear
    out = segment_mean(h, batch, 256)    # [256, 512]

The linear commutes with the segment sum, so each core only segment-
reduces its row shard of x (one-hot matmuls on the TensorEngine) and the
tiny 512x512 linear runs once per owned segment block at the end:
    out[s] = (segsum_x[s] @ W.T) * inv[s] + b * (cnt[s] > 0)
with inv/cnt host-derived from batch (metadata, like the index tensors).

batch is sorted, so core j's 16384 rows span ~33 contiguous segments and
cross-core contributions exist only at the shard boundaries, confined to
each core's first/last 2048 rows (host-asserted). The kernel therefore
processes supertiles {0, 7} first into accumulator A, extracts the <=16
boundary-partial rows with a selection matmul, and launches a tiny
AllGather (16KB/core) that completes while supertiles 1..6 stream into
accumulator B. The epilogue combines A+B+gathered partials with
selection matmuls (inv-counts folded into the selection weights) - no
big collective, no indirect scatter, ~3us serial tail.

Sharding: data-parallel over rows. x/batch split along dim 0 across 8
cores; W/b replicated; host concatenates the 8x[32, 512] outputs.
"""

import numpy as np

import concourse.bacc as bacc
import concourse.bass as bass
import concourse.mybir as mybir
import concourse.tile as tile
from concourse.bass_utils import run_bass_kernel_spmd

N_CORES = 8
N_ROWS = 131072
D = 512
N_SEG = 256
SEG_PER_CORE = N_SEG // N_CORES  # 32
RPC = N_ROWS // N_CORES          # 16384 rows per core
K = 8                            # row planes per supertile
SUP = 128 * K                    # 1024 rows per supertile
N_SUP = RPC // SUP               # 16
W_WIN = 128                      # one-hot window width (span is ~33)
EXP = 8                          # exported segments per side
NEX = 2 * EXP                    # 16 exchange slots per core

F32 = mybir.dt.float32
I32 = mybir.dt.int32
BF16 = mybir.dt.bfloat16

# supertile processing order: boundary tiles first (accumulator A),
# interior tiles after (accumulator B)
A_TILES = [0, N_SUP - 1]
B_TILES = list(range(1, N_SUP - 1))


def build_nc():
    nc = bacc.Bacc("TRN2", target_bir_lowering=False, debug=False,
                   num_devices=N_CORES)
    x = nc.dram_tensor("x", [RPC, D], F32, kind="ExternalInput")
    # batchp[p, k*N_SUP + t] = batch_rel[SUP*t + K*p + k]
    batchp = nc.dram_tensor("batchp", [128, K * N_SUP], F32,
                            kind="ExternalInput")
    iota = nc.dram_tensor("iota", [128, W_WIN], BF16, kind="ExternalInput")
    selx = nc.dram_tensor("selx", [128, NEX], BF16, kind="ExternalInput")
    sell = nc.dram_tensor("sell", [128, SEG_PER_CORE], BF16,
                          kind="ExternalInput")
    selg = nc.dram_tensor("selg", [128, SEG_PER_CORE], BF16,
                          kind="ExternalInput")
    wt = nc.dram_tensor("wt", [D, D], BF16, kind="ExternalInput")
    bindb = nc.dram_tensor("bindb", [SEG_PER_CORE, D], F32,
                           kind="ExternalInput")
    out = nc.dram_tensor("out", [SEG_PER_CORE, D], F32, kind="ExternalOutput")

    # [N_SUP, 128, K, 512]; per (t, p) the (K, 512) block is 16KB contiguous
    x_r = x.ap().rearrange("(t p k) d -> t p k d", p=128, k=K)
    RG = [list(range(N_CORES))]

    with tile.TileContext(nc) as tc:
        with tc.tile_pool(name="const", bufs=1) as const, \
             tc.tile_pool(name="dram", bufs=1, space="DRAM") as dram:
            iota_sb = const.tile([128, W_WIN], BF16, name="iota_sb")
            batch_sb = const.tile([128, K * N_SUP], F32, name="batch_sb")
            selx_sb = const.tile([128, NEX], BF16, name="selx_sb")
            sell_sb = const.tile([128, SEG_PER_CORE], BF16, name="sell_sb")
            selg_sb = const.tile([128, SEG_PER_CORE], BF16, name="selg_sb")
            wt_sb = const.tile([128, 4 * D], BF16, name="wt_sb")
            bind_sb = const.tile([SEG_PER_CORE, D], F32, name="bind_sb")
            a_sb = const.tile([128, D], BF16, name="a_sb")
            b_sb = const.tile([128, D], BF16, name="b_sb")
            ex_sb = const.tile([NEX, D], BF16, name="ex_sb")
            g_sb = const.tile([128, D], BF16, name="g_sb")
            lhsT = const.tile([128, 4 * SEG_PER_CORE], BF16, name="lhsT")
            res = const.tile([SEG_PER_CORE, D], F32, name="res")

            ag_in = dram.tile([NEX, D], BF16, name="ag_in")
            ag_out = dram.tile([N_CORES * NEX, D], BF16, name="ag_out",
                               addr_space="Shared")

            # small inputs via SWDGE; keeps the HWDGE x queues clean
            nc.gpsimd.dma_start(out=iota_sb[:, :], in_=iota[:, :])
            nc.gpsimd.dma_start(out=batch_sb[:, :], in_=batchp[:, :])
            nc.gpsimd.dma_start(out=selx_sb[:, :], in_=selx[:, :])
            nc.gpsimd.dma_start(out=sell_sb[:, :], in_=sell[:, :])
            nc.gpsimd.dma_start(out=selg_sb[:, :], in_=selg[:, :])
            for i in range(4):
                nc.gpsimd.dma_start(out=wt_sb[:, i * D:(i + 1) * D],
                                    in_=wt[i * 128:(i + 1) * 128, :])
            nc.gpsimd.dma_start(out=bind_sb[:, :], in_=bindb[:, :])

            with tc.tile_pool(name="xin", bufs=4) as xp, \
                 tc.tile_pool(name="ohp", bufs=12) as ohp, \
                 tc.tile_pool(name="pacc", bufs=1, space="PSUM") as pacc, \
                 tc.tile_pool(name="pepi", bufs=1, space="PSUM") as pepi:
                psA = pacc.tile([128, D], F32, name="psA")
                psB = pacc.tile([128, D], F32, name="psB")
                psE = pepi.tile([NEX, D], F32, name="psE")
                pt = [pepi.tile([128, SEG_PER_CORE], F32, name=f"pt{c}")
                      for c in range(4)]
                po = pepi.tile([SEG_PER_CORE, D], F32, name="po")

                n_streamed = [0]

                def stream_tile(t, ps, start, stop, split4=False):
                    xt = xp.tile([128, K, D], F32, name="xt")
                    xq = nc.sync if n_streamed[0] % 2 == 0 else nc.scalar
                    n_streamed[0] += 1
                    if split4:
                        # split the final supertile so the tail of the
                        # pipeline drains per-2-plane, not per-8-plane
                        for c in range(4):
                            q = nc.sync if c % 2 == 0 else nc.scalar
                            q.dma_start(out=xt[:, 2 * c:2 * c + 2, :],
                                        in_=x_r[t][:, 2 * c:2 * c + 2, :])
                    else:
                        xq.dma_start(out=xt[:, :, :], in_=x_r[t])
                    # little-endian f32: the high halfword of each element
                    # IS its truncated bf16 value -> free bf16 operand
                    xt_bf = xt[:, :, :].bitcast(BF16)
                    for k in range(K):
                        oh = ohp.tile([128, W_WIN], BF16, name="oh")
                        nc.vector.tensor_scalar(
                            oh[:, :], iota_sb[:, :],
                            batch_sb[:, k * N_SUP + t:k * N_SUP + t + 1],
                            None, mybir.AluOpType.is_equal)
                        nc.tensor.matmul(ps[:, :], oh[:, :],
                                         xt_bf[:, k, 1::2],
                                         start=(start and k == 0),
                                         stop=(stop and k == K - 1),
                                         skip_group_check=True)

                # phase A: boundary supertiles -> psA
                for i, t in enumerate(A_TILES):
                    stream_tile(t, psA, start=(i == 0),
                                stop=(i == len(A_TILES) - 1))

                # boundary exchange: extract export rows from A, AllGather
                nc.vector.tensor_copy(a_sb[:, :], psA[:, :])
                nc.tensor.matmul(psE[:, :], selx_sb[:, :], a_sb[:, :],
                                 start=True, stop=True,
                                 skip_group_check=True)
                nc.vector.tensor_copy(ex_sb[:, :], psE[:, :])
                nc.gpsimd.dma_start(out=ag_in[:, :], in_=ex_sb[:, :])
                nc.gpsimd.collective_compute(
                    "AllGather", mybir.AluOpType.bypass, replica_groups=RG,
                    ins=[ag_in.opt()], outs=[ag_out.opt()])
                nc.gpsimd.dma_start(out=g_sb[:, :], in_=ag_out[:, :])

                # hoist the A-part of the owned-segment selection
                for c in range(4):
                    nc.tensor.matmul(pt[c][:, :],
                                     a_sb[:, c * 128:(c + 1) * 128],
                                     sell_sb[:, :], start=True, stop=False,
                                     skip_group_check=True)

                # phase B: interior supertiles -> psB
                for i, t in enumerate(B_TILES):
                    last = i == len(B_TILES) - 1
                    stream_tile(t, psB, start=(i == 0), stop=last,
                                split4=last)
                    if t == B_TILES[-2]:
                        # gathered-part selection (g_sb ready mid-stream)
                        for c in range(4):
                            nc.tensor.matmul(
                                pt[c][:, :], g_sb[:, c * 128:(c + 1) * 128],
                                selg_sb[:, :], start=False, stop=False,
                                skip_group_check=True)

                # tail: B-part selection, transpose-combine, tiny linear
                nc.vector.tensor_copy(b_sb[:, :], psB[:, :])
                for c in range(4):
                    nc.tensor.matmul(pt[c][:, :],
                                     b_sb[:, c * 128:(c + 1) * 128],
                                     sell_sb[:, :], start=False, stop=True,
                                     skip_group_check=True)
                for c in range(4):
                    nc.vector.tensor_copy(
                        lhsT[:, c * SEG_PER_CORE:(c + 1) * SEG_PER_CORE],
                        pt[c][:, :])
                for c in range(4):
                    nc.tensor.matmul(
                        po[:, :],
                        lhsT[:, c * SEG_PER_CORE:(c + 1) * SEG_PER_CORE],
                        wt_sb[:, c * D:(c + 1) * D],
                        start=(c == 0), stop=(c == 3),
                        skip_group_check=True)
                # res = seg_mean @ Wt + b*(cnt>0)  (inv folded into sel)
                nc.vector.tensor_tensor(res[:, :], po[:, :], bind_sb[:, :],
                                        mybir.AluOpType.add)
                nc.sync.dma_start(out=out[:, :], in_=res[:, :])
    nc.compile()
    return nc


def make_in_maps(x, W, b, batch):
    x = np.asarray(x, dtype=np.float32)
    W = np.asarray(W, dtype=np.float32)
    b = np.asarray(b, dtype=np.float32)
    batch = np.asarray(batch).astype(np.int64)
    npbf = mybir.dt.np(BF16)
    wt = np.ascontiguousarray(W.T).astype(npbf)
    iota = np.tile(np.arange(W_WIN, dtype=np.float32), (128, 1)).astype(npbf)
    counts = np.bincount(batch, minlength=N_SEG).astype(np.float64)
    inv = (1.0 / np.maximum(counts, 1.0)).astype(np.float32)
    ind = (counts > 0).astype(np.float32)

    def slot_seg(j, s):
        # exchange slot s of core j: 8 segs below / above its owned block
        if s < EXP:
            return SEG_PER_CORE * j - EXP + s
        return SEG_PER_CORE * (j + 1) + (s - EXP)

    in_maps = []
    for j in range(N_CORES):
        bs = batch[j * RPC:(j + 1) * RPC]
        base, last = int(bs[0]), int(bs[-1])
        lo, hi = SEG_PER_CORE * j, SEG_PER_CORE * (j + 1)
        rel = (bs - base).astype(np.float32)
        assert rel.max() < W_WIN, f"core {j}: span {int(rel.max()) + 1}"
        assert base >= lo - EXP and last < hi + EXP, \
            f"core {j}: boundary outside +-{EXP} margin ({base}, {last})"
        # foreign-segment rows must sit in the first/last supertile
        assert np.searchsorted(bs, lo) <= SUP, f"core {j}: low margin"
        assert RPC - np.searchsorted(bs, hi) <= SUP, f"core {j}: high margin"

        planes = rel.reshape(N_SUP, 128, K)
        bp = np.empty((128, K * N_SUP), np.float32)
        for k in range(K):
            bp[:, k * N_SUP:(k + 1) * N_SUP] = planes[:, :, k].T

        selx = np.zeros((128, NEX), np.float32)
        for s in range(NEX):
            seg = slot_seg(j, s)
            if 0 <= seg < N_SEG and 0 <= seg - base < W_WIN:
                selx[seg - base, s] = 1.0
        sell = np.zeros((128, SEG_PER_CORE), np.float32)
        for u in range(SEG_PER_CORE):
            p = lo + u - base
            if 0 <= p < W_WIN:
                sell[p, u] = inv[lo + u]
        selg = np.zeros((N_CORES * NEX, SEG_PER_CORE), np.float32)
        for r in range(N_CORES):
            if r == j:
                continue
            for s in range(NEX):
                seg = slot_seg(r, s)
                if lo <= seg < hi:
                    selg[r * NEX + s, seg - lo] = inv[seg]
        bindb = b.reshape(1, D) * ind[lo:hi, None]

        in_maps.append({
            "x": np.ascontiguousarray(x[j * RPC:(j + 1) * RPC]),
            "batchp": np.ascontiguousarray(bp),
            "iota": iota,
            "selx": np.ascontiguousarray(selx.astype(npbf)),
            "sell": np.ascontiguousarray(sell.astype(npbf)),
            "selg": np.ascontiguousarray(selg.astype(npbf)),
            "wt": wt,
            "bindb": np.ascontiguousarray(bindb.astype(np.float32)),
        })
    return in_maps


_NC_CACHE = {}


def kernel(x, W, b, batch, num_segments, trace=False):
    assert int(num_segments) == N_SEG
    if "nc" not in _NC_CACHE:
        _NC_CACHE["nc"] = build_nc()
    nc = _NC_CACHE["nc"]
    in_maps = make_in_maps(x, W, b, batch)
    res = run_bass_kernel_spmd(nc, in_maps, core_ids=list(range(N_CORES)),
                               trace=trace)
    full = np.concatenate([res.results[j]["out"] for j in range(N_CORES)],
                          axis=0)
    if trace:
        return full, res
    return full
